# revision 77
# baseline (speedup 1.0000x reference)
"""Deformable-attention transformer encoder layer on 8 Trainium2 cores.

Sharding: core = (batch b = core//2, L-half = core%2). Each core computes the
full value map for its batch element (needed for sampling) and runs the rest
of the layer on its 2688-query shard.

Two-stage schedule:
  Stage 1: src/pos transposes (bf16 inputs), value GEMM -> packed bf16 maps,
  and the offset/attention math, chunk-interleaved so PE/Act/DVE/DMA overlap.
  Stage 2: 448-query chunk pipeline — per chunk the deformable sampling
  (GPSIMD gathers + DVE tap-weight multiplies + PE identity-matmul
  accumulation) is immediately followed by out-proj+LN1, FFN and LN2 for the
  previous chunk, so the Pool/DMA-heavy sampling overlaps the PE/Act-heavy
  tail instead of running as a separate phase.

Value channels are laid out (head, dh-half) interleaved: map half m holds
channels (h, m*16+dh%16) for all 8 heads, so ONE [8x16-replicated] weight
tile serves both map halves (16x instead of 32x broadcast replication) and
one idx tile serves both gathers.

The 96 (head,level,point) rows are laid out as (level,point,head). Deformable
sampling: value maps are repacked per channel as bf16 vertical pairs
((v[y,x], v[y+1,x]) in one 4-byte element) with a zero border, so one GPSIMD
ap_gather index fetches all 4 bilinear taps of a point. The 4 bf16 tap
weights fold attention/bilinear/validity; DVE applies them in one in-place
bf16 multiply and the tap/point accumulation runs on the tensor engine as
bf16 identity-matmuls into PSUM.

The idx/weight DRAM staging uses tile-pool DRAM tiles with the stage-2
readers issued from the Act DMA ring: DRAM is not hazard-tracked by the tile
framework and same-ring ordering gets elided, so cross-ring reads force real
DMA-completion semaphores (same-ring reads raced the in-flight writes).

Note: the fp32->int cast (floor) compensates for HW round-to-nearest; CoreSim
truncates instead, so CoreSim outputs diverge from hardware (hardware is the
reference vs the fp32 oracle).
"""

import sys

for _p in ("/opt/trn_rl_repo",):
    if _p not in sys.path:
        sys.path.insert(0, _p)

import ml_dtypes
import numpy as np
import concourse.bass as bass
import concourse.mybir as mybir
import concourse.tile as tile
from concourse import bacc
from concourse.bass import AP
from concourse.bass_utils import run_bass_kernel_spmd

F32 = mybir.dt.float32
MMDT = mybir.dt.float32r  # matmul operand dtype (bit-identical to f32)
BF16 = mybir.dt.bfloat16
I32 = mybir.dt.int32
I16 = mybir.dt.int16
AF = mybir.ActivationFunctionType
OP = mybir.AluOpType

BF16NP = ml_dtypes.bfloat16

B, L, C = 4, 5376, 256
NH, NL, NP = 8, 3, 4
FF = 2048
SHAPES = [(64, 64), (32, 32), (16, 16)]
LVLSTART = [0, 4096, 5120]
SHIFT = 16.0  # added to pixel coords so floor == int-trunc
LQ = L // 2
QC = 448  # stage-2 chunk
NCH = LQ // QC  # 6
QHC = 2 * QC // 16  # 56
QH2 = 2 * LQ // 16  # 336

# packed-map geometry: per level rows H+1 (y0 in [-1,H-1]), cols W+2 (x in [-1,W])
PK_BASE = []
_acc = 0
for _h, _w in SHAPES:
    PK_BASE.append(_acc)
    _acc += (_h + 1) * (_w + 2)
PKS = _acc + (_acc % 2) + 2  # even + safety pad
LVLSZ = [(h + 1) * (w + 2) for h, w in SHAPES]
# gather pieces: one per level (4 level-points), so one ap_gather covers 4 lps
# and the out free-size dominates the cost model, not the map size
PIECE_OFF = PK_BASE
PIECE_SZ = LVLSZ

# value-GEMM chunks, level aligned (tokens o..o+n inside one level)
VCHUNKS = (
    [(o, 512, 0) for o in range(0, 4096, 512)]
    + [(o, 512, 1) for o in range(4096, 5120, 512)]
    + [(5120, 256, 2)]
)


def _row_lph(r):
    # partition r -> (level, point, head) with r = (l*4+p)*8 + h
    return r // 32, (r // 8) % 4, r % 8


def _host_consts():
    W = np.zeros(96, np.float32)
    H = np.zeros(96, np.float32)
    WP2 = np.zeros(96, np.float32)
    KIDX = np.zeros(96, np.float32)
    for r in range(96):
        lvl, _, _ = _row_lph(r)
        h, w = SHAPES[lvl]
        W[r], H[r] = w, h
        WP2[r] = w + 2
        # level-relative: the gather uses a per-level slice of the packed map
        KIDX[r] = -(SHIFT - 1.0) * (w + 2) - (SHIFT - 1.0)
    c = {}
    c["CONSTS"] = np.stack(
        [
            np.full(96, SHIFT - 1.0, np.float32),  # 0: clamp lo for floor
            W + SHIFT - 1.0,                       # 1: clamp hi x
            H + SHIFT - 1.0,                       # 2: clamp hi y
            np.full(96, SHIFT - 1.5, np.float32),  # 3: mask lo (on cs)
            W + SHIFT - 0.5,                       # 4: mask hi x (on cs)
            H + SHIFT - 0.5,                       # 5: mask hi y (on cs)
            WP2,                                   # 6
            KIDX,                                  # 7
            KIDX + 1.0,                            # 8
            np.zeros(96, np.float32),              # 9 (pad)
        ],
        axis=1,
    ).astype(np.float32)
    ELX = np.zeros((3, 96), np.float32)
    ELY = np.zeros((3, 96), np.float32)
    for r in range(96):
        lvl, _, _ = _row_lph(r)
        ELX[lvl, r] = SHAPES[lvl][1]
        ELY[lvl, r] = SHAPES[lvl][0]
    c["E_LVLX"], c["E_LVLY"] = ELX, ELY
    ES = np.zeros((96, 8), np.float32)
    EE = np.zeros((8, 96), np.float32)
    for r in range(96):
        ES[r, r % 8] = 1.0
        EE[r % 8, r] = 1.0
    c["E_SUM"], c["E_EXP"] = ES, EE
    c["ONESC"] = np.ones((128, 1), np.float32)
    c["E_ONE1"] = np.ones((1, 128), np.float32)
    c["IDENT"] = np.eye(128, dtype=np.float32)
    c["IDENTB"] = np.eye(128, dtype=np.float32).astype(BF16NP)
    return c


def _perm96():
    # perm[r] = original (h,l,p) index for new row r = (l*4+p)*8 + h
    perm = np.zeros(96, np.int64)
    for r in range(96):
        lvl, p, h = _row_lph(r)
        perm[r] = h * 12 + lvl * 4 + p
    return perm


def _cperm():
    # value channel newc = m*128 + h*16 + d  <-  orig c = h*32 + m*16 + d
    perm = np.zeros(256, np.int64)
    for m in range(2):
        for h in range(8):
            for dd in range(16):
                perm[m * 128 + h * 16 + dd] = h * 32 + m * 16 + dd
    return perm


def _nchunks(n, step):
    out, i = [], 0
    while i < n:
        out.append((i, min(step, n - i)))
        i += step
    return out


DEBUG_DUMP = False


def build_program(lq=LQ, qc=QC, gelu_hw=True):
    nc = bacc.Bacc("TRN2", target_bir_lowering=False, debug=False)
    consts = _host_consts()

    hd = {}
    names = []
    def dram_in(name, shape, dt=F32):
        hd[name] = nc.dram_tensor(name, list(shape), dt, kind="ExternalInput")
        names.append(name)
        return hd[name]

    for nm, shp, dt in [
        ("src_full", (L, C), BF16), ("src_own", (lq, C), BF16),
        ("pos_own", (lq, C), BF16),
        ("ref_own", (lq, NL, 2), F32),
        ("W_val", (C, C), F32), ("b_val", (C,), F32),
        ("W_off", (C, 192), F32), ("b_off", (192,), F32),
        ("W_attn", (C, 96), F32), ("b_attn", (96,), F32),
        ("W_out", (C, C), BF16), ("b_out", (C,), F32),
        ("ln1_g", (C,), F32), ("ln1_b", (C,), F32),
        ("lin1_W", (C, FF), F32), ("lin1_b", (FF,), F32),
        ("lin2_W", (FF, C), BF16), ("lin2_b", (C,), F32),
        ("ln2_g", (C,), F32), ("ln2_b", (C,), F32),
    ]:
        dram_in(nm, shp, dt)
    for k, v in consts.items():
        dram_in(k, v.shape, BF16 if v.dtype == BF16NP else F32)
    y_own = nc.dram_tensor("y_own", [lq, C], F32, kind="ExternalOutput")
    if DEBUG_DUMP:
        for dn, shp, ddt in [("dbg_qT", (2, 128, lq), F32), ("dbg_sot", (2, 128, lq), F32),
                             ("dbg_pk", (2, 128, PKS), F32), ("dbg_outT", (2, 128, lq), BF16),
                             ("dbg_x1", (2, 128, lq), F32)]:
            hd[dn] = nc.dram_tensor(dn, list(shp), ddt, kind="ExternalOutput")

    with tile.TileContext(nc) as tc:
        with nc.allow_low_precision(reason="float32r/bf16 sampling path"):
            _body(tc, nc, hd, y_own, lq, qc, gelu_hw)
    nc.compile()
    return nc, names, consts


def _body(tc, nc, d, y_own, lq, qc, gelu_hw):
    ACT = nc.scalar
    DVE = nc.vector
    GPS = nc.gpsimd
    NQC = _nchunks(lq, qc)
    NBLK = lq // 128  # 21 own-token blocks

    # idx/weight staging lives in DRAM pool tiles accessed through tile-AP
    # views so the framework tracks the write->read hazard; raw Internal dram
    # tensors get no semaphores and the stage-2 reads race the stage-1 writes
    p_dram = tc.alloc_tile_pool(name="pdram", bufs=1, space="DRAM")
    idxd = p_dram.tile([96 * 2 * lq], I16, tag="idxd", name="idxd")
    wpd = p_dram.tile([96 * lq * 4], BF16, tag="wpd", name="wpd")
    idxd_w = idxd.rearrange("(r ql qh) -> r ql qh", ql=16, qh=2 * lq // 16)
    idxd_r = idxd.rearrange("(lp h ql qh) -> h ql lp qh", lp=12, h=8, ql=16)
    wpd_w = wpd.rearrange("(r x) -> r x", r=96)
    wpd_r = wpd.rearrange("(lp h x) -> lp h x", lp=12, h=8)
    # DRAM isn't hazard-tracked by the tile framework and DMA rings complete
    # out of order, so the stage-2 readers gate on explicit DMA-completion
    # semaphores (+16 per finished DMA) incremented by the chunk writes

    def ap(nm):
        return d[nm].ap()

    st = tc.alloc_tile_pool(name="wpool", bufs=1)

    def load(nm_or_ap, p, f, tag, dt=F32):
        src = ap(nm_or_ap) if isinstance(nm_or_ap, str) else nm_or_ap
        t = st.tile([p, f], dt, tag=tag, name=tag)
        if dt == MMDT and src.dtype != MMDT:
            src = src.bitcast(MMDT)
        nc.sync.dma_start(t[:, :], src)
        return t

    # data pools + first loads (emitted before the weight DMAs: the SP queue
    # is in-order and the transposes need src/pos first)
    p_pk = tc.alloc_tile_pool(name="ppk", bufs=1)
    pk = [p_pk.tile([128, PKS], F32, tag=f"pk{m}", name=f"pk{m}") for m in range(2)]
    # zero only the never-written border cells (x=0/W+1 columns; row 0's even
    # bf16 half, row H's odd half) instead of the whole map
    for m in range(2):
        pb = pk[m][:].bitcast(BF16)
        for lvl in range(3):
            Hl, Wl = SHAPES[lvl]
            v = pb[:, 2 * PK_BASE[lvl] : 2 * (PK_BASE[lvl] + (Hl + 1) * (Wl + 2))].rearrange(
                "p (y x two) -> p y x two", x=Wl + 2, two=2)
            GPS.memset(v[:, :, 0, :], 0.0)
            GPS.memset(v[:, :, Wl + 1, :], 0.0)
            GPS.memset(v[:, 0, 1 : Wl + 1, 0], 0.0)
            GPS.memset(v[:, Hl, 1 : Wl + 1, 1], 0.0)
    p_own = tc.alloc_tile_pool(name="pown", bufs=1)
    srcOwnT = [p_own.tile([128, lq], F32, tag=f"sot{m}", name=f"sot{m}") for m in range(2)]
    p_q = tc.alloc_tile_pool(name="pq", bufs=1)
    qT = [p_q.tile([128, lq], F32, tag=f"qT{m}", name=f"qT{m}") for m in range(2)]
    p_ld = tc.alloc_tile_pool(name="pld", bufs=1)
    srcf_t = {}
    srco_t = {}
    poso_t = {}

    def _ld_tile(cache, tag, bufs, dname, i):
        if i not in cache:
            t = p_ld.tile([128, 1792], BF16, tag=tag, name=f"{tag}{i}", bufs=bufs)
            nc.sync.dma_start(t[:], AP(d[dname], i * 7 * 128 * 256, [[256, 128], [128 * 256, 7], [1, 256]]))
            cache[i] = t
        return cache[i]

    def blk_slice(cache, tag, bufs, dname, b, k):
        t = _ld_tile(cache, tag, bufs, dname, b // 7)
        return t[:, (b % 7) * 256 + 128 * k : (b % 7) * 256 + 128 * k + 128]

    qsum_t = {}

    def _qsum_tile(i):
        # q = src_own + pos_own in token-major bf16 (DVE 2x), so the qT
        # transposes are single bf16 transposes instead of PSUM accumulation
        if i not in qsum_t:
            s = _ld_tile(srco_t, "srco", 2, "src_own", i)
            p = _ld_tile(poso_t, "poso", 2, "pos_own", i)
            t = p_ld.tile([128, 1792], BF16, tag="qsum", name=f"qsum{i}", bufs=2)
            DVE.tensor_tensor(t[:], s[:], p[:], OP.add)
            qsum_t[i] = t
        return qsum_t[i]

    def blk_qsum(b, k):
        t = _qsum_tile(b // 7)
        return t[:, (b % 7) * 256 + 128 * k : (b % 7) * 256 + 128 * k + 128]

    _qsum_tile(0)
    _qsum_tile(1)
    _ld_tile(srcf_t, "srcf", 3, "src_full", 0)

    # ---- persistent weights/consts ----
    # load order = HWDGE drain order: the offset/attn path's weights first so
    # p2 chunk 0 isn't gated on the bulk weight loads
    identB = load("IDENTB", 128, 128, "identB", dt=BF16)
    woffx = [load(AP(d["W_off"], 128 * k * 192, [[192, 128], [2, 96]]), 128, 96, f"woffx{k}", dt=MMDT) for k in range(2)]
    woffy = [load(AP(d["W_off"], 128 * k * 192 + 1, [[192, 128], [2, 96]]), 128, 96, f"woffy{k}", dt=MMDT) for k in range(2)]
    wattn = [load(ap("W_attn")[128 * k : 128 * k + 128, :], 128, 96, f"wattn{k}", dt=MMDT) for k in range(2)]
    elx = load("E_LVLX", 3, 96, "elx", dt=MMDT)
    ely = load("E_LVLY", 3, 96, "ely", dt=MMDT)
    cst = load("CONSTS", 96, 10, "cst")
    battn = load(AP(d["b_attn"], 0, [[1, 96], [1, 1]]), 96, 1, "battn")
    boffx_r = load(AP(d["b_off"], 0, [[2, 96], [1, 1]]), 96, 1, "boffxr")
    boffy_r = load(AP(d["b_off"], 1, [[2, 96], [1, 1]]), 96, 1, "boffyr")
    esum = load("E_SUM", 96, 8, "esum", dt=MMDT)
    eexp = load("E_EXP", 8, 96, "eexp", dt=MMDT)
    wvalf = [load(ap("W_val")[128 * k : 128 * k + 128, :], 128, 256, f"wvalf{k}", dt=MMDT) for k in range(2)]
    wval = [[wvalf[k][:, 128 * m : 128 * m + 128] for m in range(2)] for k in range(2)]
    bvec = lambda nm, m, tag: load(AP(d[nm], 128 * m, [[1, 128], [1, 1]]), 128, 1, tag)
    bval = [bvec("b_val", m, f"bval{m}") for m in range(2)]
    woutf = [load(ap("W_out")[128 * k : 128 * k + 128, :], 128, 256, f"woutf{k}", dt=BF16) for k in range(2)]
    wout = [[woutf[k][:, 128 * m : 128 * m + 128] for m in range(2)] for k in range(2)]
    onesc = load("ONESC", 128, 1, "onesc", dt=MMDT)
    eone1 = load("E_ONE1", 1, 128, "eone1", dt=MMDT)
    ident = load("IDENT", 128, 128, "ident", dt=MMDT)
    bout = [bvec("b_out", m, f"bout{m}") for m in range(2)]
    l1g = [bvec("ln1_g", m, f"l1g{m}") for m in range(2)]
    l1b = [bvec("ln1_b", m, f"l1b{m}") for m in range(2)]
    l2g = [bvec("ln2_g", m, f"l2g{m}") for m in range(2)]
    l2b = [bvec("ln2_b", m, f"l2b{m}") for m in range(2)]
    lin2b = [bvec("lin2_b", m, f"lin2b{m}") for m in range(2)]
    boffx = st.tile([96, 1], F32, tag="boffx", name="boffx")
    boffy = st.tile([96, 1], F32, tag="boffy", name="boffy")
    # cs = pixel_coord + SHIFT - 0.5 (so rint(cs) == floor(pixel)+SHIFT)
    DVE.tensor_scalar_add(boffx[:], boffx_r[:], SHIFT - 1.0)
    DVE.tensor_scalar_add(boffy[:], boffy_r[:], SHIFT - 1.0)
    halfc = st.tile([96, 1], F32, tag="halfc", name="halfc")
    nc.any.memset(halfc[:], 0.5)

    idr = lambda: ident[:].bitcast(MMDT)

    # ================= stage 1: transposes, value GEMM -> pk, offset/attn =================
    psA = tc.alloc_tile_pool(name="psA", bufs=1, space="PSUM")
    ps2 = tc.alloc_tile_pool(name="ps2", bufs=1, space="PSUM")
    p_tr = tc.alloc_tile_pool(name="ptr", bufs=2)

    # qT = T(src_own + pos_own) and srcOwnT = T(src_own), one pass per group
    def qt_group(g0):
        nb = min(4, NBLK - g0)
        psQ = psA.tile([128, 1024], BF16, tag="pst", name="psQ")
        psS = psA.tile([128, 1024], BF16, tag="pst2", name="psS")
        for j in range(nb):
            b = g0 + j
            for k in range(2):
                so = blk_slice(srco_t, "srco", 2, "src_own", b, k)
                qo = blk_qsum(b, k)
                nc.tensor.matmul(psS[:, 256 * j + 128 * k : 256 * j + 128 * k + 128],
                                 so, identB[:],
                                 is_transpose=True, start=True, stop=True)
                nc.tensor.matmul(psQ[:, 256 * j + 128 * k : 256 * j + 128 * k + 128],
                                 qo, identB[:],
                                 is_transpose=True, start=True, stop=True)
        psQ_v = psQ[:, : 256 * nb].rearrange("p (b2 kk c) -> p b2 kk c", kk=2, c=128)
        psS_v = psS[:, : 256 * nb].rearrange("p (b2 kk c) -> p b2 kk c", kk=2, c=128)
        for k in range(2):
            ACT.activation(qT[k][:, 128 * g0 : 128 * (g0 + nb)].bitcast(MMDT).rearrange("p (b2 c) -> p b2 c", c=128),
                           psQ_v[:, :, k, :], AF.Copy)
            ACT.activation(srcOwnT[k][:, 128 * g0 : 128 * (g0 + nb)].rearrange("p (b2 c) -> p b2 c", c=128),
                           psS_v[:, :, k, :], AF.Copy)

    # value GEMM (src_full transposed on the fly) -> packed maps
    def v_chunk(o, n, lvl):
        nb = n // 128
        psT = psA.tile([128, 1024], BF16, tag="pst", name="psT")
        for j in range(nb):
            b = (o + 128 * j) // 128
            for k in range(2):
                nc.tensor.matmul(psT[:, 256 * j + 128 * k : 256 * j + 128 * k + 128],
                                 blk_slice(srcf_t, "srcf", 3, "src_full", b, k), identB[:],
                                 is_transpose=True, start=True, stop=True)
        srcc = [p_tr.tile([128, 512], F32, tag=f"srcc{k}", name=f"srcc{k}") for k in range(2)]
        psT_v = psT[:, : 256 * nb].rearrange("p (b2 kk c) -> p b2 kk c", kk=2, c=128)
        for k in range(2):
            ACT.activation(srcc[k][:, :n].bitcast(MMDT).rearrange("p (b2 c) -> p b2 c", c=128),
                           psT_v[:, :, k, :], AF.Copy)
        Wl = SHAPES[lvl][1]
        rs2 = 2 * (Wl + 2)
        y0 = (o - LVLSTART[lvl]) // Wl
        ny = n // Wl
        for m in range(2):
            ps = psA.tile([128, 512], F32, tag="psv", name="psv", bufs=1)
            for k in range(2):
                nc.tensor.matmul(ps[:, :n], wval[k][m].bitcast(MMDT), srcc[k][:, :n].bitcast(MMDT), start=(k == 0), stop=(k == 1))
            pb = pk[m][:].bitcast(BF16)
            base2 = 2 * PK_BASE[lvl]
            v0 = pb[:, base2 + rs2 * (y0 + 1) : base2 + rs2 * (y0 + 1) + ny * rs2].rearrange(
                "p (y c two) -> p y c two", c=Wl + 2, two=2)[:, :, 1 : Wl + 1, 0]
            v1 = pb[:, base2 + rs2 * y0 : base2 + rs2 * y0 + ny * rs2].rearrange(
                "p (y c two) -> p y c two", c=Wl + 2, two=2)[:, :, 1 : Wl + 1, 1]
            ps_v = ps[:, :n].rearrange("p (y x) -> p y x", x=Wl)
            ACT.activation(v0, ps_v, AF.Identity, bias=bval[m][:])
            DVE.tensor_scalar(v1, ps_v, bval[m][:], None, OP.add)

    # ---- offset/attn math -> idx/weight DRAM (chunked) ----
    pom = tc.alloc_tile_pool(name="pom", bufs=1)
    pgm = tc.alloc_tile_pool(name="pgm", bufs=1)

    def p2_chunk(o, n):
        ps_x = ps2.tile([96, 448], F32, tag="ps_x", name="ps_x")
        ps_y = ps2.tile([96, 448], F32, tag="ps_y", name="ps_y")
        for k in range(2):
            nc.tensor.matmul(ps_x[:, :n], woffx[k][:].bitcast(MMDT), qT[k][:, o : o + n].bitcast(MMDT), start=(k == 0), stop=False)
        refx = pom.tile([3, 448], MMDT, tag="refx", name="refx")
        refy = pom.tile([3, 448], MMDT, tag="refy", name="refy")
        nc.sync.dma_start(refx[:, :n], AP(d["ref_own"], 6 * o, [[2, 3], [6, n]]).bitcast(MMDT))
        nc.sync.dma_start(refy[:, :n], AP(d["ref_own"], 6 * o + 1, [[2, 3], [6, n]]).bitcast(MMDT))
        nc.tensor.matmul(ps_x[:, :n], elx[:].bitcast(MMDT), refx[:, :n].bitcast(MMDT), start=False, stop=True)
        for k in range(2):
            nc.tensor.matmul(ps_y[:, :n], woffy[k][:].bitcast(MMDT), qT[k][:, o : o + n].bitcast(MMDT), start=(k == 0), stop=False)
        nc.tensor.matmul(ps_y[:, :n], ely[:].bitcast(MMDT), refy[:, :n].bitcast(MMDT), start=False, stop=True)

        csx = pom.tile([96, 448], F32, tag="csx", name="csx", bufs=2)
        csy = pom.tile([96, 448], F32, tag="csy", name="csy", bufs=2)
        ACT.activation(csx[:, :n], ps_x[:, :n], AF.Identity, bias=boffx[:])
        ACT.activation(csy[:, :n], ps_y[:, :n], AF.Identity, bias=boffy[:])

        ps_a = ps2.tile([96, 448], F32, tag="psar", name="ps_a")
        for k in range(2):
            nc.tensor.matmul(ps_a[:, :n], wattn[k][:].bitcast(MMDT), qT[k][:, o : o + n].bitcast(MMDT), start=(k == 0), stop=(k == 1))
        ex = pom.tile([96, 448], F32, tag="ex", name="ex", bufs=2)
        ACT.activation(ex[:, :n].bitcast(MMDT), ps_a[:, :n], AF.Exp, bias=battn[:])
        ps_d = ps2.tile([8, 448], F32, tag="ps_y", name="ps_d")
        nc.tensor.matmul(ps_d[:, :n], esum[:].bitcast(MMDT), ex[:, :n].bitcast(MMDT), start=True, stop=True)
        rec = pom.tile([8, 448], F32, tag="rec", name="rec")
        DVE.reciprocal(rec[:, :n].bitcast(MMDT), ps_d[:, :n])
        ps_r = ps2.tile([96, 448], F32, tag="psar", name="ps_r")
        nc.tensor.matmul(ps_r[:, :n], eexp[:].bitcast(MMDT), rec[:, :n].bitcast(MMDT), start=True, stop=True)
        am = pom.tile([96, 448], F32, tag="am", name="am", bufs=2)
        DVE.tensor_tensor(am[:, :n], ex[:, :n], ps_r[:, :n], OP.mult)

        # validity mask on gpsimd (idle here): am *= (in-range x) * (in-range y)
        mka = pgm.tile([96, 448], F32, tag="mka", name="mka")
        mkb = pgm.tile([96, 448], F32, tag="mkb", name="mkb")
        GPS.tensor_scalar(mka[:, :n], csx[:, :n], cst[:, 3:4], None, OP.is_ge)
        GPS.tensor_scalar(mkb[:, :n], csx[:, :n], cst[:, 4:5], None, OP.is_lt)
        GPS.tensor_tensor(mka[:, :n], mka[:, :n], mkb[:, :n], OP.mult)
        GPS.tensor_scalar(mkb[:, :n], csy[:, :n], cst[:, 3:4], None, OP.is_ge)
        GPS.tensor_tensor(mka[:, :n], mka[:, :n], mkb[:, :n], OP.mult)
        GPS.tensor_scalar(mkb[:, :n], csy[:, :n], cst[:, 5:6], None, OP.is_lt)
        GPS.tensor_tensor(mka[:, :n], mka[:, :n], mkb[:, :n], OP.mult)
        DVE.tensor_tensor(am[:, :n], am[:, :n], mka[:, :n], OP.mult)

        ti = pom.tile([96, 448], I32, tag="ti", name="ti")
        fx = pom.tile([96, 448], F32, tag="fx", name="fx", bufs=2)
        fy = pom.tile([96, 448], F32, tag="fy", name="fy", bufs=2)
        wfx = pom.tile([96, 448], F32, tag="wfx", name="wfx", bufs=2)
        wfy = pom.tile([96, 448], F32, tag="wfy", name="wfy", bufs=2)
        for (coord, tt, fl, wf, hi_clamp) in ((csx, ti, fx, wfx, 1), (csy, ti, fy, wfy, 2)):
            # HW fp32->int32 cast rounds to nearest; rint(cs) == floor(pixel)+SHIFT
            DVE.tensor_copy(tt[:, :n], coord[:, :n])
            # fused i32->f32 convert + clamp (clamp changes the floor only for
            # masked points, so the weight wf below may use the clamped value)
            GPS.tensor_scalar(fl[:, :n], tt[:, :n], cst[:, 0:1], cst[:, hi_clamp : hi_clamp + 1], OP.max, OP.min)
            DVE.tensor_tensor(wf[:, :n], coord[:, :n], fl[:, :n], OP.subtract)  # wx-0.5
        tmp = pom.tile([96, 448], F32, tag="tmp", name="tmp", bufs=2)
        DVE.scalar_tensor_tensor(tmp[:, :n], fy[:, :n], cst[:, 6:7], fx[:, :n], OP.mult, OP.add)
        # interleaved (left, right) indices, i16, q_lo-swizzled for the 16-wrap:
        # flat position Q = 2q+r; iu2[p, ql, qh] = index for Q = qh*16+ql
        iu2 = pom.tile([96, 2 * 448], I16, tag="iu2", name="iu2")
        nqh2 = 2 * n // 16
        iu2_v = iu2[:, : 2 * n].rearrange("p (ql qh) -> p ql qh", ql=16)
        tmp_v = tmp[:, :n].rearrange("p (qh m) -> p m qh", m=8)
        DVE.tensor_scalar(iu2_v[:, 0::2, :], tmp_v, cst[:, 7:8], None, OP.add)
        DVE.tensor_scalar(iu2_v[:, 1::2, :], tmp_v, cst[:, 8:9], None, OP.add)
        nc.sync.dma_start(idxd_w[:, :, 2 * o // 16 : 2 * o // 16 + nqh2], iu2_v)

        wxa = pom.tile([96, 448], F32, tag="wxa", name="wxa")
        wxb = pom.tile([96, 448], F32, tag="wxb", name="wxb")
        wya = pom.tile([96, 448], F32, tag="wya", name="wya")
        wyb = pom.tile([96, 448], F32, tag="wyb", name="wyb")
        ACT.activation(wxa[:, :n], wfx[:, :n], AF.Identity, bias=halfc[:], scale=-1.0)  # 1-wx
        ACT.activation(wxb[:, :n], wfx[:, :n], AF.Identity, bias=halfc[:], scale=1.0)   # wx
        ACT.activation(wya[:, :n], wfy[:, :n], AF.Identity, bias=halfc[:], scale=-1.0)
        ACT.activation(wyb[:, :n], wfy[:, :n], AF.Identity, bias=halfc[:], scale=1.0)
        atop, abot = wya, wyb
        DVE.tensor_tensor(atop[:, :n], am[:, :n], wya[:, :n], OP.mult)
        DVE.tensor_tensor(abot[:, :n], am[:, :n], wyb[:, :n], OP.mult)
        wp = pom.tile([96, 448, 4], BF16, tag="wp", name="wp")
        DVE.tensor_tensor(wp[:, :n, 0], atop[:, :n], wxa[:, :n], OP.mult)
        DVE.tensor_tensor(wp[:, :n, 1], abot[:, :n], wxa[:, :n], OP.mult)
        DVE.tensor_tensor(wp[:, :n, 2], atop[:, :n], wxb[:, :n], OP.mult)
        DVE.tensor_tensor(wp[:, :n, 3], abot[:, :n], wxb[:, :n], OP.mult)
        nc.sync.dma_start(wpd_w[:, 4 * o : 4 * o + 4 * n],
                          wp[:, :n, :].rearrange("p a b -> p (a b)"))

    # qT group g completes queries < 512(g+1); p2 chunk c needs < 448(c+1)
    qt_group(0)
    qt_group(4)
    vi = 0
    for ci in range(len(NQC)):
        g0 = 8 + 4 * ci
        if g0 < NBLK:
            qt_group(g0)
        p2_chunk(*NQC[ci])
        # a couple of value-GEMM chunks between p2 chunks keeps PE fed
        for _ in range(2):
            if vi < len(VCHUNKS):
                v_chunk(*VCHUNKS[vi])
                vi += 1
    while vi < len(VCHUNKS):
        v_chunk(*VCHUNKS[vi])
        vi += 1
    pgm.release()
    pom.release()
    p_tr.release()
    ps2.release()
    psA.release()
    p_ld.release()
    if DEBUG_DUMP:
        for m in range(2):
            nc.sync.dma_start(AP(d["dbg_qT"], m * 128 * lq, [[lq, 128], [1, lq]]), qT[m][:])
            nc.sync.dma_start(AP(d["dbg_sot"], m * 128 * lq, [[lq, 128], [1, lq]]), srcOwnT[m][:])
            nc.sync.dma_start(AP(d["dbg_pk"], m * 128 * PKS, [[PKS, 128], [1, PKS]]), pk[m][:])
    p_q.release()

    # ================= stage 2: chunked sampling + encoder tail pipeline =================
    psP = tc.alloc_tile_pool(name="psP", bufs=1, space="PSUM")
    p_smp = tc.alloc_tile_pool(name="psmp", bufs=1)
    p_out = tc.alloc_tile_pool(name="pout", bufs=1)
    p_ffn = tc.alloc_tile_pool(name="pffn", bufs=1)
    p_fw = tc.alloc_tile_pool(name="pfw", bufs=1)
    outT = [p_out.tile([128, lq], BF16, tag=f"outT{m}", name=f"outT{m}") for m in range(2)]
    x1 = [p_out.tile([128, lq], F32, tag=f"x1{m}", name=f"x1{m}") for m in range(2)]
    lneps = p_out.tile([1, 1], F32, tag="lneps", name="lneps")
    nc.any.memset(lneps[:], 1e-5)
    lin1 = lin2 = lin1b = None
    qhc = 2 * qc // 16

    def samp_start(c):
        idxc = p_smp.tile([128, 12 * qhc], I16, tag="idxc", name="idxc", bufs=2)
        nc.scalar.dma_start(idxc[:, :], idxd_r[:, :, :, c * qhc : (c + 1) * qhc])
        accs = [psP.tile([128, qc], F32, tag=f"acc{m}", name=f"acc{m}") for m in range(2)]
        return idxc, accs

    def samp_piece(c, piece, idxc, accs):
        o = c * qc
        wts = []
        for j in range(4):
            lp = piece * 4 + j
            wt = p_smp.tile([128, 4 * qc], BF16, tag="wt", name="wt", bufs=3)
            nc.scalar.dma_start(wt[:, :], wpd_r[lp][:, 4 * o : 4 * o + 4 * qc]
                              .unsqueeze(1).broadcast_to((8, 16, 4 * qc)))
            wts.append(wt)
        for m in range(2):
            gt = p_smp.tile([128, 4, qc, 2], F32, tag="gt", name="gt", bufs=3)
            GPS.ap_gather(gt[:].rearrange("p a b c -> p (a b c)"),
                          pk[m][:, PIECE_OFF[piece] : PIECE_OFF[piece] + PIECE_SZ[piece]],
                          idxc[:, piece * 4 * qhc : (piece + 1) * 4 * qhc],
                          channels=128, num_elems=PIECE_SZ[piece], d=1, num_idxs=8 * qc)
            for j in range(4):
                gb = gt[:, j].bitcast(BF16).rearrange("p a b -> p (a b)")
                DVE.tensor_tensor(gb, gb, wts[j][:, :], OP.mult)
            pv = gt[:].bitcast(BF16)  # [128, 4, qc, 4] bf16 taps
            for j in range(4):
                for tp in range(4):
                    nc.tensor.matmul(accs[m][:, :], identB[:], pv[:, j, :, tp],
                                     start=(piece == 0 and j == 0 and tp == 0),
                                     stop=(piece == 2 and j == 3 and tp == 3))

    def samp_end(c, accs):
        o = c * qc
        for m in range(2):
            ACT.activation(outT[m][:, o : o + qc], accs[m][:, :], AF.Copy)

    def _ln_tail(xs, g, bb, dst, o, n):
        ps_s = psP.tile([1, qc], F32, tag="S", name="ln_s1", bufs=2)
        ps_s2 = psP.tile([1, qc], F32, tag="S", name="ln_s2", bufs=2)
        sq = p_ffn.tile([128, qc], F32, tag="ln_sq", name="ln_sq")
        for m in range(2):
            nc.tensor.matmul(ps_s[:, :n], onesc[:].bitcast(MMDT), xs[m].bitcast(MMDT), start=(m == 0), stop=(m == 1))
        for m in range(2):
            ACT.activation(sq[:, :n].bitcast(MMDT), xs[m], AF.Square)
            nc.tensor.matmul(ps_s2[:, :n], onesc[:].bitcast(MMDT), sq[:, :n].bitcast(MMDT), start=(m == 0), stop=(m == 1))
        mean = p_ffn.tile([1, qc], F32, tag="ln_mean", name="ln_mean")
        ACT.activation(mean[:, :n].bitcast(MMDT), ps_s[:, :n], AF.Copy, scale=1.0 / 256.0)
        ms = p_ffn.tile([1, qc], F32, tag="ln_ms", name="ln_ms")
        DVE.tensor_tensor(ms[:, :n], mean[:, :n], mean[:, :n], OP.mult)
        vpe = p_ffn.tile([1, qc], F32, tag="ln_vpe", name="ln_vpe")
        DVE.scalar_tensor_tensor(vpe[:, :n], ps_s2[:, :n], 1.0 / 256.0, ms[:, :n], OP.mult, OP.subtract)
        std = p_ffn.tile([1, qc], F32, tag="ln_std", name="ln_std")
        ACT.activation(std[:, :n], vpe[:, :n], AF.Sqrt, bias=lneps[:])
        rstd = p_ffn.tile([1, qc], F32, tag="ln_rstd", name="ln_rstd")
        DVE.reciprocal(rstd[:, :n].bitcast(MMDT), std[:, :n])
        ps_bm = psP.tile([128, qc], F32, tag="H2a", name="ln_bm")
        ps_br = psP.tile([128, qc], F32, tag="H2b", name="ln_br")
        nc.tensor.matmul(ps_bm[:, :n], eone1[:].bitcast(MMDT), mean[:, :n].bitcast(MMDT), start=True, stop=True)
        nc.tensor.matmul(ps_br[:, :n], eone1[:].bitcast(MMDT), rstd[:, :n].bitcast(MMDT), start=True, stop=True)
        for m in range(2):
            t = p_ffn.tile([128, qc], F32, tag="ln_t", name="ln_t")
            DVE.tensor_tensor(t[:, :n], xs[m], ps_bm[:, :n], OP.subtract)
            DVE.tensor_tensor(t[:, :n], t[:, :n], ps_br[:, :n], OP.mult)
            ACT.activation(dst[m][:, o : o + n].bitcast(MMDT), t[:, :n], AF.Identity, bias=bb[m][:], scale=g[m][:])

    def ln1_chunk(c):
        o, n = c * qc, qc
        xs = []
        for m in range(2):
            ps = psP.tile([128, qc], F32, tag="H1", name="ln_ps", bufs=2)
            for k in range(2):
                nc.tensor.matmul(ps[:, :n], wout[k][m], outT[k][:, o : o + n], start=(k == 0), stop=(k == 1))
            x = p_ffn.tile([128, qc], F32, tag=f"ln_x{m}", name=f"ln_x{m}")
            ACT.activation(x[:, :n].bitcast(MMDT), ps[:, :n], AF.Identity, bias=bout[m][:])
            DVE.tensor_tensor(x[:, :n].bitcast(MMDT), x[:, :n], srcOwnT[m][:, o : o + n], OP.add)
            xs.append(x[:, :n])
        _ln_tail(xs, l1g, l1b, x1, o, n)

    ffn_state = {}

    def ffn_half(c, half):
        o, n = c * qc, qc
        if half == 0:
            ffn_state["h2"] = [psP.tile([128, qc], F32, tag=f"H2{'ab'[m]}", name=f"ps_h2{m}") for m in range(2)]
        ps_h2 = ffn_state["h2"]
        for mf in range(8 * half, 8 * half + 8):
            ps_h1 = psP.tile([128, qc], F32, tag="H1", name="ps_h1", bufs=2)
            for k in range(2):
                nc.tensor.matmul(ps_h1[:, :n], lin1[k][:, 128 * mf : 128 * mf + 128].bitcast(MMDT), x1[k][:, o : o + n].bitcast(MMDT), start=(k == 0), stop=(k == 1))
            h1 = p_ffn.tile([128, qc], BF16, tag="h1", name="h1", bufs=2)
            ACT.activation(h1[:, :n], ps_h1[:, :n], AF.Gelu if gelu_hw else AF.Tanh, bias=lin1b[:, mf : mf + 1])
            for m in range(2):
                nc.tensor.matmul(ps_h2[m][:, :n], lin2[:, 256 * mf + 128 * m : 256 * mf + 128 * m + 128], h1[:, :n], start=(mf == 0), stop=(mf == 15))
        if half == 1:
            for m in range(2):
                DVE.scalar_tensor_tensor(x1[m][:, o : o + n].bitcast(MMDT), ps_h2[m][:, :n],
                                         lin2b[m][:], x1[m][:, o : o + n], OP.add, OP.add)

    def ln2_chunk(c):
        o, n = c * qc, qc
        xs = [x1[m][:, o : o + n] for m in range(2)]
        _ln_tail(xs, l2g, l2b, x1, o, n)

    def emit_y(b0, nb):
        # transpose nb (<=2) 128-token blocks back to token-major and store
        psY = psP.tile([128, 512], F32, tag="H1", name="psY", bufs=2)
        for j in range(nb):
            for m in range(2):
                nc.tensor.matmul(psY[:, 256 * j + 128 * m : 256 * j + 128 * m + 128].bitcast(MMDT),
                                 x1[m][:, 128 * (b0 + j) : 128 * (b0 + j) + 128].bitcast(MMDT), idr(),
                                 is_transpose=True, start=True, stop=True)
        yq = p_ffn.tile([128, 512], F32, tag="yq", name="yq", bufs=2)
        ACT.activation(yq[:, : 256 * nb], psY[:, : 256 * nb], AF.Copy)
        # y stores ride the Act queue so they never head-of-line-block the
        # SP queue's idx/weight loads for later chunks
        nc.scalar.dma_start(AP(y_own, b0 * 128 * 256, [[256, 128], [128 * 256, nb], [1, 256]]),
                            yq[:, : 256 * nb].rearrange("p (b c) -> p b c", c=256))

    yb = 0
    prev = None
    # sampling for chunk c is interleaved piece-wise with the previous
    # chunk's out-proj/LN/FFN so every engine's in-order stream alternates
    # between gather-chain work and ready tail work
    for c in range(NCH):
        idxc, accs = samp_start(c)
        last = c == NCH - 1
        if last and prev is not None:
            # final window: the previous chunk's LN1+FFN go ahead of the
            # sampling pieces; its LN2/y interleave with chunk 5's tail below
            ln1_chunk(prev)
            ffn_half(prev, 0)
            ffn_half(prev, 1)
        if not last and prev is not None:
            ln1_chunk(prev)
        samp_piece(c, 0, idxc, accs)
        if not last and prev is not None:
            ffn_half(prev, 0)
        samp_piece(c, 1, idxc, accs)
        if not last and prev is not None:
            ffn_half(prev, 1)
        samp_piece(c, 2, idxc, accs)
        samp_end(c, accs)
        if not last and prev is not None:
            ln2_chunk(prev)
            while (yb + 2) * 128 <= qc * (prev + 1):
                emit_y(yb, 2)
                yb += 2
        if c == 0:
            lin1 = [p_fw.tile([128, FF], MMDT, tag=f"lin1{k}", name=f"lin1{k}") for k in range(2)]
            for k in range(2):
                nc.sync.dma_start(lin1[k][:, :], ap("lin1_W")[128 * k : 128 * k + 128, :].bitcast(MMDT))
            lin2 = p_fw.tile([128, 16 * 256], BF16, tag="lin2", name="lin2")
            nc.sync.dma_start(lin2[:, :], AP(d["lin2_W"], 0, [[256, 128], [128 * 256, 16], [1, 256]]))
            lin1b = p_fw.tile([128, 16], F32, tag="lin1b", name="lin1b")
            nc.sync.dma_start(lin1b[:, :], AP(d["lin1_b"], 0, [[1, 128], [128, 16]]))
        prev = c
    # drain: interleave chunk 4's LN2/y with chunk 5's tail so the two
    # latency-bound LN chains overlap instead of running back to back
    ln2_chunk(prev - 1)
    ln1_chunk(prev)
    while (yb + 2) * 128 <= qc * prev:
        emit_y(yb, 2)
        yb += 2
    ffn_half(prev, 0)
    ffn_half(prev, 1)
    ln2_chunk(prev)
    while yb < NBLK:
        nb = min(2, NBLK - yb)
        emit_y(yb, nb)
        yb += nb
    if DEBUG_DUMP:
        for m in range(2):
            nc.sync.dma_start(AP(d["dbg_outT"], m * 128 * lq, [[lq, 128], [1, lq]]), outT[m][:])
            nc.sync.dma_start(AP(d["dbg_x1"], m * 128 * lq, [[lq, 128], [1, lq]]), x1[m][:])

    p_fw.release()
    p_ffn.release()
    p_out.release()
    p_smp.release()
    psP.release()
    p_own.release()
    p_pk.release()
    p_dram.release()
    st.release()


# ======================= host side =======================

_CACHE = {}


def _get_program():
    if "nc" not in _CACHE:
        nc, names, consts = build_program()
        _CACHE["nc"] = nc
        _CACHE["names"] = names
        _CACHE["consts"] = consts
    return _CACHE["nc"], _CACHE["names"], _CACHE["consts"]


def make_in_maps(inputs, lq=LQ, consts=None):
    consts = consts if consts is not None else _host_consts()
    src = np.ascontiguousarray(np.asarray(inputs["src"], np.float32))
    pos = np.ascontiguousarray(np.asarray(inputs["pos"], np.float32))
    ref = np.ascontiguousarray(np.asarray(inputs["reference_points"], np.float32))
    weights = {
        k: np.ascontiguousarray(np.asarray(inputs[k]), np.float32)
        for k in ["W_val", "b_val", "W_off", "b_off", "W_attn", "b_attn",
                   "W_out", "b_out", "ln1_g", "ln1_b", "lin1_W", "lin1_b",
                   "lin2_W", "lin2_b", "ln2_g", "ln2_b"]
    }
    # permute the 96 (h,l,p) rows into (l,p,h) order
    perm = _perm96()
    perm2 = np.stack([2 * perm, 2 * perm + 1], axis=1).reshape(-1)
    weights["W_off"] = np.ascontiguousarray(weights["W_off"][:, perm2])
    weights["b_off"] = np.ascontiguousarray(weights["b_off"][perm2])
    weights["W_attn"] = np.ascontiguousarray(weights["W_attn"][:, perm])
    weights["b_attn"] = np.ascontiguousarray(weights["b_attn"][perm])
    # value channels (head, dh-half) interleaved so one weight/idx tile
    # serves both map halves
    cp = _cperm()
    weights["W_val"] = np.ascontiguousarray(weights["W_val"][:, cp])
    weights["b_val"] = np.ascontiguousarray(weights["b_val"][cp])
    weights["W_out"] = np.ascontiguousarray(weights["W_out"][cp, :]).astype(BF16NP)
    weights["lin2_W"] = weights["lin2_W"].astype(BF16NP)
    src_bf = src.astype(BF16NP)
    pos_bf = pos.astype(BF16NP)
    in_maps = []
    for core in range(8):
        b, half = core // 2, core % 2
        o = half * lq
        m = {
            "src_full": src_bf[b],
            "src_own": np.ascontiguousarray(src_bf[b, o : o + lq]),
            "pos_own": np.ascontiguousarray(pos_bf[b, o : o + lq]),
            "ref_own": np.ascontiguousarray(ref[b, o : o + lq]),
        }
        m.update(weights)
        m.update(consts)
        in_maps.append(m)
    return in_maps


def kernel(**inputs):
    nc, names, consts = _get_program()
    in_maps = make_in_maps(inputs, consts=consts)
    res = run_bass_kernel_spmd(nc, in_maps, core_ids=list(range(8)))
    out = np.zeros((B, L, C), np.float32)
    for core in range(8):
        b, half = core // 2, core % 2
        o = half * LQ
        out[b, o : o + LQ] = res.results[core]["y_own"]
    return out
